# revision 5
# baseline (speedup 1.0000x reference)
"""Cost-volume concatenation kernel for Trainium2 (8 NeuronCores).

Reference (per batch b, disparity index d, i = d + MIN_DISP):
  out[b, d, h, w, 0:C]  = left[b, h, w, :]    if 0 <= w - i < W else 0
  out[b, d, h, w, C:2C] = right[b, h, w-i, :] if 0 <= w - i < W else 0

Sharding: disparity-parallel, interleaved -- core c builds disparities
{8j + c : j in 0..15} for the full [B, H, W] volume.  Interleaving
balances valid-span widths (bytes written) across cores.

SPMD trick: run_bass_kernel_spmd runs ONE program on all 8 cores, so the
per-core disparity offset c cannot appear in any access pattern.  The
program is written for i0 = 8j - 112 and all c-dependence is moved into
the data:
  * rightp input = right pre-shifted by +c columns and zero-padded to
    W+8 columns (rightp[x] = right[x-c], 0 outside) -- so the program's
    static gather rightp[w - i0] yields right[w - i] with the out-of-
    range mask already applied by the padding.
  * mask input = per-column 1.0/0.0 validity (mask[x] = 1 iff c <= x <
    W+c), expanded to [H, (W+8)*C]; the left half is left * mask.
Each plane writes the union-over-c of valid w-spans; columns inside the
union but outside the core's true span receive exact zeros from the
padding/mask, columns outside the union are never written and rely on
ExternalOutput buffers being pre-zeroed (bass2jax donates zero buffers
to PJRT for exactly this purpose).

On-chip, the interleaved [left | shifted right] rows are built per plane
in an SBUF tile [96 (h), W*2C]: ScalarE copies the right half, VectorE
multiplies the left half by the mask (both full-96-partition chunked
copies), and one DMA stores the union w-span (0.9-2.3 MB contiguous-ish).
"""

import os
import sys

sys.path.insert(0, "/opt/trn_rl_repo")

import numpy as np

B, H, W, C = 2, 96, 192, 16
D = 128
MIN_DISP = -112
N_CORES = 8
DPC = D // N_CORES         # 16 disparity planes per core
PAD = 8                    # right/mask padded to W + PAD columns
WP = W + PAD
COLS = W * 2 * C           # 6144 interleaved f32 per (b,d,h) row

_CACHE = {}


def _plane_span(j):
    """Union-over-c valid w-span for plane j (program-static)."""
    i0 = 8 * j + MIN_DISP
    if i0 < 0:
        us, ue = 0, min(W + i0 + (N_CORES - 1), W)
    else:
        us, ue = i0, W
    return i0, us, ue


def _build_program():
    from concourse import bacc, mybir
    import concourse.tile as tile

    nc = bacc.Bacc(
        "TRN2", target_bir_lowering=False, debug=False, num_devices=N_CORES
    )
    f32 = mybir.dt.float32
    left = nc.dram_tensor("left", [B, H, W * C], f32, kind="ExternalInput")
    rightp = nc.dram_tensor("rightp", [B, H, WP * C], f32, kind="ExternalInput")
    mask = nc.dram_tensor("mask", [H, WP * C], f32, kind="ExternalInput")
    out = nc.dram_tensor("out", [B, DPC, H, COLS], f32, kind="ExternalOutput")

    with tile.TileContext(nc) as tc:
        with (
            tc.tile_pool(name="inputs", bufs=1) as ipool,
            tc.tile_pool(name="work", bufs=4) as wpool,
        ):
            lsb, rsb = [], []
            for b in range(B):
                lt = ipool.tile([H, W * C], f32, tag=f"l{b}")
                nc.sync.dma_start(lt[:, :], left.ap()[b])
                lsb.append(lt)
                rt = ipool.tile([H, WP * C], f32, tag=f"r{b}")
                nc.sync.dma_start(rt[:, :], rightp.ap()[b])
                rsb.append(rt)
            msb = ipool.tile([H, WP * C], f32, tag="mask")
            nc.sync.dma_start(msb[:, :], mask.ap())

            store_engines = [nc.sync, nc.scalar]
            n = 0
            for b in range(B):
                for j in range(DPC):
                    i0, us, ue = _plane_span(j)
                    nw = ue - us
                    x0 = us - i0          # source column offset into rightp/mask

                    T = wpool.tile([H, COLS], f32, tag="out")
                    t_chunk = T[:, us * 32 : ue * 32].rearrange(
                        "p (w c) -> p w c", c=32
                    )
                    src_r = rsb[b][:, (x0) * C : (x0 + nw) * C].rearrange(
                        "p (w c) -> p w c", c=C
                    )
                    src_l = lsb[b][:, us * C : ue * C].rearrange(
                        "p (w c) -> p w c", c=C
                    )
                    src_m = msb[:, (x0) * C : (x0 + nw) * C].rearrange(
                        "p (w c) -> p w c", c=C
                    )
                    nc.scalar.copy(t_chunk[:, :, C : 2 * C], src_r)
                    nc.vector.tensor_mul(t_chunk[:, :, 0:C], src_l, src_m)

                    dst = out.ap()[b, j, :, us * 32 : ue * 32]
                    store_engines[n % 2].dma_start(dst, T[:, us * 32 : ue * 32])
                    n += 1

    nc.compile()
    return nc


def _get_program():
    if "nc" not in _CACHE:
        _CACHE["nc"] = _build_program()
    return _CACHE["nc"]


def kernel(left, right):
    from concourse.bass_utils import run_bass_kernel_spmd

    left = np.ascontiguousarray(left, dtype=np.float32).reshape(B, H, W * C)
    right = np.ascontiguousarray(right, dtype=np.float32)
    nc = _get_program()

    in_maps = []
    for c in range(N_CORES):
        rp = np.zeros((B, H, WP, C), dtype=np.float32)
        rp[:, :, c : c + W] = right
        m = np.zeros((WP, 1), dtype=np.float32)
        m[c : c + W] = 1.0
        mfull = np.broadcast_to(
            m.reshape(1, WP, 1), (H, WP, C)
        ).reshape(H, WP * C)
        in_maps.append(
            {
                "left": left,
                "rightp": rp.reshape(B, H, WP * C),
                "mask": np.ascontiguousarray(mfull),
            }
        )

    prof_dir = os.environ.get("BASS_NTFF_DIR")
    if prof_dir:
        from trn_agent_boot.trn_boot import _ntff_profile_via_ctypes

        hook = _ntff_profile_via_ctypes("/opt/axon/libaxon_pjrt.so")
        with hook(prof_dir, [0]):
            res = run_bass_kernel_spmd(nc, in_maps, core_ids=list(range(N_CORES)))
    else:
        res = run_bass_kernel_spmd(nc, in_maps, core_ids=list(range(N_CORES)))

    # parts[c][b, j] is disparity d = 8j + c -> stack on a new axis after j.
    parts = [
        res.results[c]["out"].reshape(B, DPC, H, W, 2 * C)
        for c in range(N_CORES)
    ]
    return np.stack(parts, axis=2).reshape(B, D, H, W, 2 * C)


# revision 7
# speedup vs baseline: 94739.7525x; 94739.7525x over previous
"""Cost-volume concatenation kernel for Trainium2 (8 NeuronCores).

Reference (per batch b, disparity index d, i = d + MIN_DISP):
  out[b, d, h, w, 0:C]  = left[b, h, w, :]    if 0 <= w - i < W else 0
  out[b, d, h, w, C:2C] = right[b, h, w-i, :] if 0 <= w - i < W else 0

Sharding: disparity-parallel, interleaved -- core c builds disparities
{8j + c : j in 0..15} for the full [B, H, W] volume.  Interleaving
balances valid-span widths (bytes written) across cores.

SPMD trick: run_bass_kernel_spmd runs ONE program on all 8 cores, so the
per-core offset c cannot appear in any access pattern.  The program is
written for i0 = 8j - 112 and all c-dependence lives in the data:
  * rightp input = right pre-shifted by +c columns, zero-padded to W+8
    columns -- the program's static gather rightp[w - i0] then yields
    right[w - i] with the out-of-range mask applied by the padding.
  * cvec input = per-partition scalars [16c, 16(W+c)]; the left-half
    validity mask (left is zeroed outside the valid span) is built
    on-chip: mask[x] = (iota(x) >= 16c) * (iota(x) < 16(W+c)) over
    expanded columns x = 16*w_src + ch.
Each plane writes the union-over-c of valid w-spans; columns inside the
union but outside the core's true span receive exact zeros from the
padding/mask; columns outside the union are never written and rely on
ExternalOutput buffers being pre-zeroed (bass2jax donates zero buffers
to PJRT for exactly this purpose).

Tiles: one disparity plane per SBUF tile, 96 h-rows.  Consecutive planes
are staggered by 32 partitions (even -> rows 0:96, odd -> rows 32:128)
and stored on the two HWDGE rings (sync/scalar): a lone 96-partition DMA
only engages 12 of the 16 SBUF AXI ports (~250 GB/s measured); two
staggered concurrent stores cover all 16 (~330 GB/s measured for 128p).
ScalarE copies the right half, VectorE multiplies the left half by the
mask; one DMA per plane stores the union w-span (0.9-2.3 MB).
"""

import os
import sys

sys.path.insert(0, "/opt/trn_rl_repo")

import numpy as np

B, H, W, C = 2, 96, 192, 16
D = 128
MIN_DISP = -112
N_CORES = 8
DPC = D // N_CORES         # 16 disparity planes per core
PAD = 8                    # rightp padded to W + PAD source columns
WP = W + PAD
COLS = W * 2 * C           # 6144 interleaved f32 per (b,d,h) row

_CACHE = {}


def _plane_span(j):
    """Union-over-c valid w-span for plane j (program-static)."""
    i0 = 8 * j + MIN_DISP
    if i0 < 0:
        us, ue = 0, min(W + i0 + (N_CORES - 1), W)
    else:
        us, ue = i0, W
    return i0, us, ue


def _build_program():
    from concourse import bacc, mybir
    import concourse.tile as tile

    nc = bacc.Bacc(
        "TRN2", target_bir_lowering=False, debug=False, num_devices=N_CORES
    )
    f32 = mybir.dt.float32
    left = nc.dram_tensor("left", [B, H, W * C], f32, kind="ExternalInput")
    rightp = nc.dram_tensor("rightp", [B, H, WP * C], f32, kind="ExternalInput")
    cvec = nc.dram_tensor("cvec", [128, 2], f32, kind="ExternalInput")
    out = nc.dram_tensor("out", [B, DPC, H, COLS], f32, kind="ExternalOutput")

    with tile.TileContext(nc) as tc:
        with (
            tc.tile_pool(name="inputs", bufs=1) as ipool,
            tc.tile_pool(name="work", bufs=3) as wpool,
        ):
            # Input tiles, two stagger phases: phase 0 data at rows 0:96,
            # phase 1 at rows 32:128.
            lsb = {}   # (b, phase) -> (tile, row0)
            rsb = {}
            for b in range(B):
                for ph in range(2):
                    r0 = 32 * ph
                    lt = ipool.tile([128, W * C], f32, tag=f"l{b}{ph}")
                    rt = ipool.tile([128, WP * C], f32, tag=f"r{b}{ph}")
                    lsb[(b, ph)] = (lt, r0)
                    rsb[(b, ph)] = (rt, r0)

            # Loads: phase-0 b=0 on the SWDGE queue; phase-1 b=0 at the
            # head of the two HWDGE store rings; b=1 behind on SWDGE.
            nc.gpsimd.dma_start(lsb[(0, 0)][0][0:96, :], left.ap()[0])
            nc.gpsimd.dma_start(rsb[(0, 0)][0][0:96, :], rightp.ap()[0])
            nc.sync.dma_start(lsb[(0, 1)][0][32:128, :], left.ap()[0])
            nc.scalar.dma_start(rsb[(0, 1)][0][32:128, :], rightp.ap()[0])
            cv = ipool.tile([128, 2], f32, tag="cvec")
            nc.gpsimd.dma_start(cv[:, :], cvec.ap())
            for b2 in range(1, B):
                nc.gpsimd.dma_start(lsb[(b2, 0)][0][0:96, :], left.ap()[b2])
                nc.gpsimd.dma_start(rsb[(b2, 0)][0][0:96, :], rightp.ap()[b2])
                nc.gpsimd.dma_start(lsb[(b2, 1)][0][32:128, :], left.ap()[b2])
                nc.gpsimd.dma_start(rsb[(b2, 1)][0][32:128, :], rightp.ap()[b2])

            # Mask over expanded source columns x = 16*w_src + ch,
            # identical on every partition: 1.0 iff 16c <= x < 16(W+c).
            xio = ipool.tile([128, WP * C], f32, tag="xio")
            msk = ipool.tile([128, WP * C], f32, tag="msk")
            nc.gpsimd.iota(
                xio[:, :], [[1, WP * C]], channel_multiplier=0,
                allow_small_or_imprecise_dtypes=True,
            )
            nc.vector.tensor_single_scalar(
                msk[:, :], xio[:, :], cv[:, 0:1], mybir.AluOpType.is_ge
            )
            nc.vector.tensor_single_scalar(
                xio[:, :], xio[:, :], cv[:, 1:2], mybir.AluOpType.is_lt
            )
            nc.vector.tensor_mul(msk[:, :], msk[:, :], xio[:, :])

            store_engines = [nc.sync, nc.scalar]
            for n in range(B * DPC):
                b, j = divmod(n, DPC)
                ph = n % 2
                i0, us, ue = _plane_span(j)
                nw = ue - us
                x0 = us - i0      # source column offset into rightp/mask

                lt, r0 = lsb[(b, ph)]
                rt, _ = rsb[(b, ph)]
                T = wpool.tile([128, COLS], f32, tag="out")
                # Compute-engine APs must fit the naturally-aligned
                # partition block of their size, so the 32-offset phase
                # is split into [32:64) + [64:128) ops.
                segs = [(32, 32), (64, 64)] if r0 == 32 else [(0, 96)]
                for s0, sn in segs:
                    s1 = s0 + sn
                    t_chunk = T[s0:s1, us * 32 : ue * 32].rearrange(
                        "p (w c) -> p w c", c=32
                    )
                    src_r = rt[s0:s1, x0 * C : (x0 + nw) * C].rearrange(
                        "p (w c) -> p w c", c=C
                    )
                    src_l = lt[s0:s1, us * C : ue * C].rearrange(
                        "p (w c) -> p w c", c=C
                    )
                    src_m = msk[s0:s1, x0 * C : (x0 + nw) * C].rearrange(
                        "p (w c) -> p w c", c=C
                    )
                    nc.scalar.copy(t_chunk[:, :, C : 2 * C], src_r)
                    nc.vector.tensor_mul(t_chunk[:, :, 0:C], src_l, src_m)

                dst = out.ap()[b, j, :, us * 32 : ue * 32]
                store_engines[ph].dma_start(
                    dst, T[r0 : r0 + H, us * 32 : ue * 32]
                )

    nc.compile()
    return nc


def _get_program():
    if "nc" not in _CACHE:
        _CACHE["nc"] = _build_program()
    return _CACHE["nc"]


def kernel(left, right):
    from concourse.bass_utils import run_bass_kernel_spmd

    left = np.ascontiguousarray(left, dtype=np.float32).reshape(B, H, W * C)
    right = np.ascontiguousarray(right, dtype=np.float32)
    nc = _get_program()

    in_maps = []
    for c in range(N_CORES):
        rp = np.zeros((B, H, WP, C), dtype=np.float32)
        rp[:, :, c : c + W] = right
        cv = np.empty((128, 2), dtype=np.float32)
        cv[:, 0] = 16.0 * c
        cv[:, 1] = 16.0 * (W + c)
        in_maps.append(
            {
                "left": left,
                "rightp": rp.reshape(B, H, WP * C),
                "cvec": cv,
            }
        )

    prof_dir = os.environ.get("BASS_NTFF_DIR")
    if prof_dir:
        from trn_agent_boot.trn_boot import _ntff_profile_via_ctypes

        hook = _ntff_profile_via_ctypes("/opt/axon/libaxon_pjrt.so")
        with hook(prof_dir, [0]):
            res = run_bass_kernel_spmd(nc, in_maps, core_ids=list(range(N_CORES)))
    else:
        res = run_bass_kernel_spmd(nc, in_maps, core_ids=list(range(N_CORES)))

    # parts[c][b, j] is disparity d = 8j + c -> stack on a new axis after j.
    parts = [
        res.results[c]["out"].reshape(B, DPC, H, W, 2 * C)
        for c in range(N_CORES)
    ]
    return np.stack(parts, axis=2).reshape(B, D, H, W, 2 * C)
